# revision 18
# baseline (speedup 1.0000x reference)
"""Deformable conv (B=4, C=256, H=W=64, O=256, K=3, pad=1) on 8 NeuronCores.

Sharding: core = (image b, row-half h): each core computes out[b, :, h*32:(h+1)*32, :].

Host precomputes (free): the gather indices (int16) and the 4 bilinear corner
slot-weights (fp16, exact f64 math incl. boundary shift cases), plus a
row-pair duplicated image zt[y, x] = (x[:, y, x], x[:, y+1, x]) in fp16 so a
single 2KB gather descriptor fetches all 4 corners (TL, BL, TR, BR) of one
(position, tap) sample.

Per-core device pipeline:
  1. GpSimd dma_gathers 2KB corner rows from zt: 2 calls (512+640 idx) per
     128-position chunk; NBUF-deep buffer rotation.
  2. Per chunk the 9 taps split into two combine paths (load balance):
     - PE-diag taps: DVE builds diag(w_corner) tiles (id * w); PE fuses
       transpose + 4-corner weighted sum as 4 accumulating fp16 matmuls
       g_tile @ diag(w) into fp32 PSUM.
     - DVE-comb taps: DVE computes the weighted 4-corner sum in gather
       layout (per-partition scalar weights), PE transposes with a single
       identity matmul per 128-ch tile.
  3. Act copies PSUM tiles into the fp16 cols buffer; PE runs the
     O=256 x CK=2304 x P=2048 GEMM with fp32 PSUM accumulation; Act copies
     to fp16 out_sb; DMA stores (host widens to f32).

Execution: _make_exec builds a cached jitted shard_map executor (same
_bass_exec_p lowering as bass2jax.run_bass_via_pjrt) so repeated calls skip
re-trace/re-lower/re-compile.
"""

import numpy as np

B, C, H, W = 4, 256, 64, 64
O, KK = 256, 9
HW = H * W                      # 4096
ROWS_PER_CORE = H // 2          # 32
P_CORE = ROWS_PER_CORE * W      # 2048 output positions per core
N_PB = P_CORE // 128            # 16 chunks (position blocks)
N_IDX = N_PB * KK * 128         # 18432
NG = N_IDX // 16                # 1152
N_SLOT = N_PB * KK              # 144
N_CKT = 2 * KK                  # 18 ck-tiles of 128
GEMM_NP = 512                   # positions per GEMM pass
N_GP = P_CORE // GEMM_NP        # 4 GEMM passes
N_CORES = 8
NZT = H * (W + 1)               # 4160 zt entries of 512 fp16 (1KB)

DVE_TAPS = (2, 4, 6)            # taps combined on DVE (rest use PE diag path)
ACT_TAPS = (0, 8)               # PE-diag taps whose diag builds run on Act

_CACHE = {}


def _build_nc(reps=1, skip=()):
    skip = set(skip)
    import concourse.bacc as bacc
    import concourse.mybir as mybir
    from concourse import library_config
    import bass_rust

    F32, F16, I16 = mybir.dt.float32, mybir.dt.float16, mybir.dt.int16
    AF = mybir.ActivationFunctionType
    AL = mybir.AluOpType

    nc = bacc.Bacc("TRN2", num_swdge_queues=4)

    # ---- DRAM parameters (per-core inputs) ----
    zt = nc.declare_dram_parameter("zt", [(NZT + 1) * 2 * C], F16, isOutput=False)
    wt = nc.declare_dram_parameter("wt", [N_CKT, 128, O], F16, isOutput=False)
    idx_d = nc.declare_dram_parameter("idx", [128, NG], I16, isOutput=False)
    wtl_d = nc.declare_dram_parameter("wtl", [128, N_SLOT], F32, isOutput=False)
    wtr_d = nc.declare_dram_parameter("wtr", [128, N_SLOT], F32, isOutput=False)
    wbl_d = nc.declare_dram_parameter("wbl", [128, N_SLOT], F32, isOutput=False)
    wbr_d = nc.declare_dram_parameter("wbr", [128, N_SLOT], F32, isOutput=False)
    ident = nc.declare_dram_parameter("ident", [128, 128], F16, isOutput=False)
    out = nc.declare_dram_parameter("out", [O, P_CORE], F16, isOutput=True)

    from contextlib import ExitStack
    st = ExitStack()
    sb = lambda n, s, d: st.enter_context(nc.sbuf_tensor(n, s, d))
    ps = lambda n, s, d: st.enter_context(nc.psum_tensor(n, s, d))

    # ---- SBUF ----
    idx_sb = sb("idx_sb", [128, NG], I16)
    wtl = sb("wtl_sb", [128, N_SLOT], F32)
    wtr = sb("wtr_sb", [128, N_SLOT], F32)
    wbl = sb("wbl_sb", [128, N_SLOT], F32)
    wbr = sb("wbr_sb", [128, N_SLOT], F32)
    # transposed cols, full [128, N_CKT, P_CORE] fp16
    cols = sb("cols", [128, N_CKT, P_CORE], F16)
    # weights in SBUF [128, N_CKT, O] fp16
    wt_sb = sb("wt_sb", [128, N_CKT, O], F16)
    id_sb = sb("id_sb", [128, 128], F16)
    out_sb = [sb(f"out_sb{i}", [128, GEMM_NP], F16) for i in range(2)]

    # gather buffers (NBUF-deep): [128, KK, 1024] fp16 (4 corners x 256ch)
    NBUF = 4
    g_buf = [sb(f"g_buf{i}", [128, KK, 1024], F16) for i in range(NBUF)]
    # per-(corner, tap) diagonal weight matrices, NDB-deep
    NDB = 3
    dbuf = [sb(f"dbuf{i}", [128, 4, KK, 128], F16) for i in range(NDB)]
    # DVE-combined tiles [pos, dve-tap, 256ch], NDB-deep
    NDT = len(DVE_TAPS)
    comb = [sb(f"comb{i}", [128, NDT, 256], F16) for i in range(NDB)]

    NPST = 4
    ps_t = [ps(f"ps_t{i}", [128, 4, 128], F32) for i in range(NPST)]
    ps_g = [ps(f"ps_g{i}", [128, GEMM_NP], F32) for i in range(2)]

    N_LOADS = 5                  # wtl/wtr/wbl/wbr/ident (idx, wt tracked apart)
    IO_ALL = N_LOADS * 16
    W_OFFS = {0: 0, 1: 512, 2: 256, 3: 768}   # j2 -> corner offset (TL,TR,BL,BR)

    def emit_body():
        s_io = nc.alloc_semaphore("s_io")
        s_iow = nc.alloc_semaphore("s_iow")
        s_ix = nc.alloc_semaphore("s_ix")
        s_g = [nc.alloc_semaphore(f"s_g{i}") for i in range(NBUF)]
        s_dg = nc.alloc_semaphore("s_dg")
        s_db = nc.alloc_semaphore("s_db")
        s_tr = nc.alloc_semaphore("s_tr")
        s_cpy = nc.alloc_semaphore("s_cpy")
        s_mm = nc.alloc_semaphore("s_mm")
        s_oc = nc.alloc_semaphore("s_oc")
        s_st = [nc.alloc_semaphore(f"s_st{i}") for i in range(2)]
        blk_cm = nc.Block()
        block = blk_cm.__enter__()

        @block.sync
        def _(sync):
            sync.dma_start(idx_sb[:], idx_d[:]).then_inc(s_ix, 16)
            sync.dma_start(wt_sb[:], wt.rearrange("t c e -> c t e")).then_inc(s_iow, 16)
            for src, dst in [(wtl_d, wtl), (wtr_d, wtr), (wbl_d, wbl),
                             (wbr_d, wbr), (ident, id_sb)]:
                sync.dma_start(dst[:], src[:]).then_inc(s_io, 16)
            for g in range(N_GP):
                for ob in range(2):
                    j = 2 * g + ob
                    sync.wait_ge(s_oc, j + 1)
                    sync.dma_start(out[ob * 128:(ob + 1) * 128,
                                       g * GEMM_NP:(g + 1) * GEMM_NP],
                                   out_sb[ob][:]).then_inc(s_st[ob], 16)
            sync.wait_ge(s_st[0], 16 * N_GP)
            sync.wait_ge(s_st[1], 16 * N_GP)

        @block.vector
        def _(vector):
            vector.wait_ge(s_io, IO_ALL)
            for c in range(N_PB):
                if c >= NDB:
                    vector.wait_ge(s_tr, 5 * (c - NDB + 1))
                if "combine" in skip:
                    vector.drain().then_inc(s_dg, 1)
                    continue
                vector.wait_ge(s_g[c % NBUF], 32 * (c // NBUF) + 16)
                d = dbuf[c % NDB]
                cbf = comb[c % NDB]
                gg = g_buf[c % NBUF]
                s0 = KK * c
                # sweep A: first corner of DVE taps + all diag builds
                for ki, k in enumerate(DVE_TAPS):
                    vector.tensor_scalar(cbf[:, ki, :], gg[:, k, 0:256],
                                         wtl[:, s0 + k:s0 + k + 1], None, AL.mult)
                for k in range(KK):
                    if k in DVE_TAPS or k in ACT_TAPS:
                        continue
                    for j2, w_ in ((0, wtl), (1, wtr), (2, wbl), (3, wbr)):
                        vector.tensor_scalar(d[:, j2, k, :], id_sb[:],
                                             w_[:, s0 + k:s0 + k + 1], None, AL.mult)
                for off, w_ in ((512, wtr), (256, wbl), (768, wbr)):
                    for ki, k in enumerate(DVE_TAPS):
                        vector.scalar_tensor_tensor(
                            cbf[:, ki, :], gg[:, k, off:off + 256],
                            w_[:, s0 + k:s0 + k + 1], cbf[:, ki, :],
                            AL.mult, AL.add)
                vector.drain().then_inc(s_dg, 1)

        @block.gpsimd
        def _(gpsimd):
            gpsimd.load_library(library_config.mlp)
            gpsimd.wait_ge(s_ix, 16)
            zt_view = zt[:].copy()
            zt_view.ap = bass_rust.VecI64Pair([[2 * C, NZT], [1, 4 * C]])
            for c in range(N_PB):
                if c >= NBUF:
                    gpsimd.wait_ge(s_tr, 5 * (c - NBUF + 1))
                i0 = c * 72
                # one dma_gather tops out at 1024 idx -> slot-aligned split
                # 1152 = 1024 + 128 (taps 0-7, then tap 8); the DVE combine
                # only needs taps <8, so it can proceed after the first call.
                for qi, (lo, hi, sl0, sl1) in enumerate(((0, 64, 0, 8),
                                                        (64, 72, 8, 9))):
                    nidx = (hi - lo) * 16
                    if "gather" in skip:
                        gpsimd.sem_inc(s_g[c % NBUF], 16)
                        continue
                    # both calls of a chunk share one queue so this sem's
                    # increments arrive in call order (per-queue FIFO)
                    gpsimd.dma_gather(g_buf[c % NBUF][:, sl0:sl1, :], zt_view,
                                      idx_sb[:, i0 + lo:i0 + hi],
                                      nidx, nidx, 4 * C, elem_step=2 * C,
                                      queue_num=c % 4).then_inc(
                                          s_g[c % NBUF], 16)

        @block.tensor
        def _(tensor):
            tensor.wait_ge(s_io, IO_ALL)
            tensor.wait_ge(s_iow, 16)
            for c in range(N_PB):
                tensor.wait_ge(s_dg, c + 1)
                tensor.wait_ge(s_db, c + 1)
                tensor.wait_ge(s_g[c % NBUF], 32 * (c // NBUF + 1))
                gg = g_buf[c % NBUF]
                d = dbuf[c % NDB]
                cbf = comb[c % NDB]
                for k in range(KK):
                    for cb in range(2):
                        t = 2 * k + cb
                        g4, q = t // 4, t % 4
                        jj = c * 5 + g4          # global 4-group index
                        if "pe" in skip:
                            if q == 0:
                                tensor.sem_inc(s_tr, 1)
                            continue
                        if q == 0 and jj >= NPST:
                            tensor.wait_ge(s_cpy, jj - NPST + 1)
                        if k in DVE_TAPS:
                            ki = DVE_TAPS.index(k)
                            mm = tensor.matmul(ps_t[jj % NPST][:, q, :],
                                               cbf[:, ki, cb * 128:(cb + 1) * 128],
                                               id_sb[:], start=True, stop=True)
                        else:
                            for ci in range(4):
                                src = gg[:, k, W_OFFS[ci] + cb * 128:
                                         W_OFFS[ci] + (cb + 1) * 128]
                                mm = tensor.matmul(ps_t[jj % NPST][:, q, :], src,
                                                   d[:, ci, k, :],
                                                   start=(ci == 0), stop=(ci == 3))
                        if q == 3 or t == N_CKT - 1:
                            mm.then_inc(s_tr, 1)
                if c % 4 == 3:
                    g = c // 4
                    tensor.wait_ge(s_cpy, 5 * (c + 1))
                    for ob in range(2):
                        if "pe" in skip:
                            tensor.sem_inc(s_mm, 1)
                            continue
                        if g >= 1:
                            tensor.wait_ge(s_oc, 2 * (g - 1) + ob + 1)
                        for t in range(N_CKT):
                            mm = tensor.matmul(
                                ps_g[ob][:],
                                wt_sb[:, t, ob * 128:(ob + 1) * 128],
                                cols[:, t, g * GEMM_NP:(g + 1) * GEMM_NP],
                                start=(t == 0), stop=(t == N_CKT - 1))
                        mm.then_inc(s_mm, 1)

        @block.scalar
        def _(scalar):
            scalar.wait_ge(s_io, IO_ALL)
            for c in range(N_PB):
                if "act" in skip:
                    scalar.sem_inc(s_db, 1)
                else:
                    d = dbuf[c % NDB]
                    s0 = KK * c
                    for ti_, k in enumerate(ACT_TAPS):
                        for j2, w_ in ((0, wtl), (1, wtr), (2, wbl), (3, wbr)):
                            a = scalar.activation(d[:, j2, k, :], id_sb[:],
                                                  AF.Copy,
                                                  scale=w_[:, s0 + k:s0 + k + 1])
                    a.then_inc(s_db, 1)
                for g4 in range(5):
                    nt = 4 if g4 < 4 else 2      # tiles in this group (18 = 4*4+2)
                    jj = c * 5 + g4
                    if "act" in skip:
                        scalar.sem_inc(s_cpy, 1)
                        continue
                    scalar.wait_ge(s_tr, jj + 1)
                    t0_ = g4 * 4
                    scalar.activation(cols[:, t0_:t0_ + nt, c * 128:(c + 1) * 128],
                                      ps_t[jj % NPST][:, 0:nt, :],
                                      AF.Copy).then_inc(s_cpy, 1)
                if c % 4 == 3:
                    g = c // 4
                    for ob in range(2):
                        jj = 2 * g + ob
                        if "act" in skip:
                            scalar.sem_inc(s_oc, 1)
                            continue
                        scalar.wait_ge(s_mm, jj + 1)
                        if g >= 1:
                            scalar.wait_ge(s_st[ob], 16 * g)
                        scalar.activation(out_sb[ob][:], ps_g[ob][:],
                                          AF.Copy).then_inc(s_oc, 1)

        blk_cm.__exit__(None, None, None)

    snap = nc._state.snapshot_sems()
    for rep in range(reps):
        emit_body()
        if rep < reps - 1:
            nc.clear_and_free_semaphores(nc._state.allocated_since(snap))
            nc.all_engine_barrier()
            nc._state.restore_sems(snap)

    st.close()
    nc.compile()
    return nc


def _to_w(a, dtype):
    # w-layout [128, N_SLOT]: (part, KK*pb + k) = val[k, pb*128 + part]
    return np.ascontiguousarray(
        a.reshape(KK, N_PB, 128).transpose(2, 1, 0)).reshape(128, N_SLOT).astype(dtype)


def _to_g(a, dtype):
    # g-layout [128, NG]: (16g + l, 72*pb + 8*k + w) = val[k, pb*128 + w*16 + l]
    a4 = a.reshape(KK, N_PB, 8, 16)              # [k, pb, w, l]
    g1 = np.ascontiguousarray(a4.transpose(3, 1, 0, 2)).reshape(16, NG)
    return np.tile(g1, (8, 1)).astype(dtype)     # replicate to 128 partitions


def _host_prep(x, offset, weight):
    """Build the 8 per-core input maps."""
    f16 = np.float16
    # row-pair duplicated image zt[y, x] = (x[:, y, x], x[:, y+1, x]), fp16,
    # with a zero column at x=W and zero second half at y=H-1.
    zts = []
    for b in range(B):
        t = np.ascontiguousarray(x[b].transpose(1, 2, 0)).astype(f16)  # [H, W, C]
        zrow = np.zeros((H, W + 1, 2 * C), dtype=f16)
        zrow[:, :W, :C] = t
        zrow[:H - 1, :W, C:] = t[1:]
        ztf = np.zeros(((NZT + 1) * 2 * C,), dtype=f16)
        ztf[:NZT * 2 * C] = zrow.reshape(-1)
        zts.append(ztf)
    # weights: wt[t, c, o] = weight[o, cb*128+c, ky, kx],  t = 2*(3*ky+kx) + cb
    wr = weight.reshape(O, C, KK).transpose(2, 1, 0)   # [KK, C, O]
    wt = np.empty((N_CKT, 128, O), dtype=f16)
    for k in range(KK):
        for cb in range(2):
            wt[2 * k + cb] = wr[k, cb * 128:(cb + 1) * 128, :].astype(f16)

    ident = np.eye(128, dtype=f16)

    ky, kx = np.meshgrid(np.arange(3), np.arange(3), indexing="ij")
    ky = ky.reshape(-1).astype(np.float64)   # [KK]
    kx = kx.reshape(-1).astype(np.float64)

    in_maps = []
    p = np.arange(P_CORE)
    for core in range(N_CORES):
        b, hhalf = core // 2, core % 2
        i0 = hhalf * ROWS_PER_CORE
        off = offset[b].reshape(KK, 2, H, W)[:, :, i0:i0 + ROWS_PER_CORE, :]
        offy = off[:, 0].reshape(KK, P_CORE).astype(np.float64)
        offx = off[:, 1].reshape(KK, P_CORE).astype(np.float64)
        py = (i0 + p // W - 1)[None, :] + ky[:, None] + offy   # [KK, P]
        px = (p % W - 1)[None, :] + kx[:, None] + offx
        y0 = np.floor(py)
        x0 = np.floor(px)
        dy = py - y0
        dx = px - x0
        # slot weights: boundary-shifted so the 2x2 window at the clipped
        # entry (yc, xc) carries exactly the valid reference corner weights
        vy0 = (y0 >= 0) & (y0 <= H - 1)
        vy1 = (y0 + 1 >= 0) & (y0 + 1 <= H - 1)
        wy0 = (1.0 - dy) * vy0 + dy * (vy1 & (y0 == -1))
        wy1 = dy * (vy1 & (y0 != -1))
        vx0 = (x0 >= 0) & (x0 <= W - 1)
        vx1 = (x0 + 1 >= 0) & (x0 + 1 <= W - 1)
        wx0 = (1.0 - dx) * vx0 + dx * (vx1 & (x0 == -1))
        wx1 = dx * (vx1 & (x0 != -1))
        yc = np.clip(y0, 0, H - 1).astype(np.int64)
        xc = np.clip(x0, 0, W - 1).astype(np.int64)
        idx = (yc * (W + 1) + xc).astype(np.int16)

        in_maps.append({
            "zt": zts[b], "wt": wt, "ident": ident,
            "idx": _to_g(idx, np.int16),
            "wtl": _to_w(wy0 * wx0, np.float32), "wtr": _to_w(wy0 * wx1, np.float32),
            "wbl": _to_w(wy1 * wx0, np.float32), "wbr": _to_w(wy1 * wx1, np.float32),
        })
    return in_maps


def _assemble(results):
    out = np.empty((B, O, H, W), dtype=np.float32)
    for core in range(N_CORES):
        b, hhalf = core // 2, core % 2
        i0 = hhalf * ROWS_PER_CORE
        out[b, :, i0:i0 + ROWS_PER_CORE, :] = \
            np.asarray(results[core]["out"]).astype(np.float32).reshape(
                O, ROWS_PER_CORE, W)
    return out


def _make_exec(nc, donate=False):
    """Build a cached jitted SPMD executor for a compiled Bass module.

    Replicates concourse.bass2jax.run_bass_via_pjrt's lowering (same
    _bass_exec_p bind / shard_map layout) but returns a reusable jitted
    callable, so repeated invocations skip re-trace/re-lower/re-compile.
    """
    import jax
    import numpy as _np
    from jax.sharding import Mesh, PartitionSpec
    from jax.experimental.shard_map import shard_map
    from concourse import bass2jax
    import concourse.mybir as mybir

    bass2jax.install_neuronx_cc_hook()
    assert nc.dbg_addr is None
    partition_name = (nc.partition_id_tensor.name
                      if nc.partition_id_tensor else None)

    in_names, out_names, out_avals, zero_outs = [], [], [], []
    for alloc in nc.m.functions[0].allocations:
        if not isinstance(alloc, mybir.MemoryLocationSet):
            continue
        name = alloc.memorylocations[0].name
        if alloc.kind == "ExternalInput":
            if name != partition_name:
                in_names.append(name)
        elif alloc.kind == "ExternalOutput":
            out_names.append(name)
            shape = tuple(alloc.tensor_shape)
            dtype = mybir.dt.np(alloc.dtype)
            out_avals.append(jax.core.ShapedArray(shape, dtype))
            zero_outs.append(_np.zeros(shape, dtype))
    n_params = len(in_names)
    all_names = list(in_names) + list(out_names)
    if partition_name is not None:
        all_names.append(partition_name)
    all_names = tuple(all_names)

    def _body(*args):
        operands = list(args)
        if partition_name is not None:
            operands.append(bass2jax.partition_id_tensor())
        outs = bass2jax._bass_exec_p.bind(
            *operands,
            out_avals=tuple(out_avals),
            in_names=all_names,
            out_names=tuple(out_names),
            lowering_input_output_aliases=(),
            sim_require_finite=True,
            sim_require_nnan=True,
            nc=nc,
        )
        return tuple(outs)

    devices = jax.devices()[:N_CORES]
    mesh = Mesh(np.asarray(devices), ("core",))
    n_out = len(out_names)
    fn = jax.jit(
        shard_map(_body, mesh=mesh,
                  in_specs=(PartitionSpec("core"),) * (n_params + n_out),
                  out_specs=(PartitionSpec("core"),) * n_out,
                  check_rep=False),
        donate_argnums=tuple(range(n_params, n_params + n_out)) if donate else (),
        keep_unused=True,
    )
    return {"fn": fn, "in_names": in_names, "out_names": out_names,
            "zero_outs": zero_outs, "mesh": mesh, "n_params": n_params}


def _concat_inputs(ex, in_maps):
    return [np.concatenate([in_maps[c][n] for c in range(N_CORES)], axis=0)
            for n in ex["in_names"]]


def _concat_zeros(ex):
    return [np.zeros((N_CORES * z.shape[0], *z.shape[1:]), z.dtype)
            for z in ex["zero_outs"]]


def kernel(x, offset, weight):
    x = np.asarray(x, dtype=np.float32)
    offset = np.asarray(offset, dtype=np.float32)
    weight = np.asarray(weight, dtype=np.float32)
    if "nc" not in _CACHE:
        _CACHE["nc"] = _build_nc()
    if "exec" not in _CACHE:
        _CACHE["exec"] = _make_exec(_CACHE["nc"])
    ex = _CACHE["exec"]
    in_maps = _host_prep(x, offset, weight)
    outs = ex["fn"](*_concat_inputs(ex, in_maps), *_concat_zeros(ex))
    full = np.asarray(outs[0]).reshape(N_CORES, O, P_CORE)
    results = [{"out": full[c]} for c in range(N_CORES)]
    return _assemble(results)


# revision 20
# speedup vs baseline: 1.1583x; 1.1583x over previous
"""Deformable conv (B=4, C=256, H=W=64, O=256, K=3, pad=1) on 8 NeuronCores.

Sharding: core = (image b, row-half h): each core computes out[b, :, h*32:(h+1)*32, :].

Host precomputes (free): the gather indices (int16) and the 4 bilinear corner
slot-weights (exact f64 math incl. boundary shift cases), plus a row-pair
duplicated image zt[y, x] = (x[:, y, x], x[:, y+1, x]) in fp16 so a single
2KB gather descriptor fetches all 4 corners (TL, BL, TR, BR) of one
(position, tap) sample.

Per-core device pipeline (per 128-position chunk, 16 chunks):
  1. GpSimd dma_gathers 2KB corner rows from zt: 2 SWDGE calls (512+640 idx)
     per chunk, queues rotating per chunk; 4-deep buffer ring.
  2. The 9 taps split across three combine paths (engine load balance; DVE
     and the SWDGE descriptor generator contend for the shared SBUF port
     pair, so DVE work is sized to share the budget with the gather DGE):
     - PE-diag taps {1,3,5,7}: DVE builds diag(w_corner) tiles (id * w);
       PE fuses transpose + 4-corner weighted sum as 4 accumulating fp16
       matmuls g_tile @ diag(w) into fp32 PSUM.
     - PE-diag taps {0,8}: same, but the diag builds run on the Scalar
       (Act) engine (activation copy with per-partition scale) which has
       its own SBUF ports.
     - DVE-comb taps {2,4,6}: DVE computes the weighted 4-corner sum in
       gather layout (per-partition scalar weights, ts + 3 stt chain), PE
       transposes with a single identity matmul per 128-ch tile.
  3. Act copies PSUM tiles into the fp16 cols buffer; PE runs the
     O=256 x CK=2304 x P=2048 GEMM (fp32 PSUM accumulation, N=512 passes);
     Act copies to fp16 out_sb; DMA stores (host widens to f32).

Execution: _make_exec builds a cached jitted shard_map executor (same
_bass_exec_p lowering as bass2jax.run_bass_via_pjrt) so repeated calls skip
re-trace/re-lower/re-compile.
"""

import numpy as np

B, C, H, W = 4, 256, 64, 64
O, KK = 256, 9
HW = H * W                      # 4096
ROWS_PER_CORE = H // 2          # 32
P_CORE = ROWS_PER_CORE * W      # 2048 output positions per core
N_PB = P_CORE // 128            # 16 chunks (position blocks)
N_IDX = N_PB * KK * 128         # 18432
NG = N_IDX // 16                # 1152
N_SLOT = N_PB * KK              # 144
N_CKT = 2 * KK                  # 18 ck-tiles of 128
GEMM_NP = 512                   # positions per GEMM pass
N_GP = P_CORE // GEMM_NP        # 4 GEMM passes
N_CORES = 8
NZT = H * (W + 1)               # 4160 zt entries of 512 fp16 (1KB)

DVE_TAPS = (2, 4, 6)            # taps combined on DVE (rest use PE diag path)
ACT_TAPS = (0, 8)               # PE-diag taps whose diag builds run on Act

_CACHE = {}


def _build_nc(reps=1, skip=()):
    skip = set(skip)
    import concourse.bacc as bacc
    import concourse.mybir as mybir
    from concourse import library_config
    import bass_rust

    F32, F16, I16 = mybir.dt.float32, mybir.dt.float16, mybir.dt.int16
    AF = mybir.ActivationFunctionType
    AL = mybir.AluOpType

    nc = bacc.Bacc("TRN2", num_swdge_queues=4)

    # ---- DRAM parameters (per-core inputs) ----
    zt = nc.declare_dram_parameter("zt", [(NZT + 1) * 2 * C], F16, isOutput=False)
    wt = nc.declare_dram_parameter("wt", [N_CKT, 128, O], F16, isOutput=False)
    idx_d = nc.declare_dram_parameter("idx", [128, NG], I16, isOutput=False)
    wtl_d = nc.declare_dram_parameter("wtl", [128, N_SLOT], F32, isOutput=False)
    wtr_d = nc.declare_dram_parameter("wtr", [128, N_SLOT], F32, isOutput=False)
    wbl_d = nc.declare_dram_parameter("wbl", [128, N_SLOT], F32, isOutput=False)
    wbr_d = nc.declare_dram_parameter("wbr", [128, N_SLOT], F32, isOutput=False)
    ident = nc.declare_dram_parameter("ident", [128, 128], F16, isOutput=False)
    out = nc.declare_dram_parameter("out", [O, P_CORE], F16, isOutput=True)

    from contextlib import ExitStack
    st = ExitStack()
    sb = lambda n, s, d: st.enter_context(nc.sbuf_tensor(n, s, d))
    ps = lambda n, s, d: st.enter_context(nc.psum_tensor(n, s, d))

    # ---- SBUF ----
    idx_sb = sb("idx_sb", [128, NG], I16)
    wtl = sb("wtl_sb", [128, N_SLOT], F32)
    wtr = sb("wtr_sb", [128, N_SLOT], F32)
    wbl = sb("wbl_sb", [128, N_SLOT], F32)
    wbr = sb("wbr_sb", [128, N_SLOT], F32)
    # transposed cols, full [128, N_CKT, P_CORE] fp16
    cols = sb("cols", [128, N_CKT, P_CORE], F16)
    # weights in SBUF [128, N_CKT, O] fp16
    wt_sb = sb("wt_sb", [128, N_CKT, O], F16)
    id_sb = sb("id_sb", [128, 128], F16)
    out_sb = [sb(f"out_sb{i}", [128, GEMM_NP], F16) for i in range(2)]

    # gather buffers (NBUF-deep): [128, KK, 1024] fp16 (4 corners x 256ch)
    NBUF = 4
    g_buf = [sb(f"g_buf{i}", [128, KK, 1024], F16) for i in range(NBUF)]
    # per-(corner, tap) diagonal weight matrices, NDB-deep
    NDB = 3
    dbuf = [sb(f"dbuf{i}", [128, 4, KK, 128], F16) for i in range(NDB)]
    # DVE-combined tiles [pos, dve-tap, 256ch], NDB-deep
    NDT = len(DVE_TAPS)
    comb = [sb(f"comb{i}", [128, NDT, 256], F16) for i in range(NDB)]

    NPST = 4
    ps_t = [ps(f"ps_t{i}", [128, 4, 128], F32) for i in range(NPST)]
    ps_g = [ps(f"ps_g{i}", [128, GEMM_NP], F32) for i in range(2)]

    N_LOADS = 5                  # wtl/wtr/wbl/wbr/ident (idx, wt tracked apart)
    IO_ALL = N_LOADS * 16
    W_OFFS = {0: 0, 1: 512, 2: 256, 3: 768}   # j2 -> corner offset (TL,TR,BL,BR)

    def emit_body():
        s_io = nc.alloc_semaphore("s_io")
        s_iow = nc.alloc_semaphore("s_iow")
        s_ix = nc.alloc_semaphore("s_ix")
        s_g = [nc.alloc_semaphore(f"s_g{i}") for i in range(NBUF)]
        s_dg = nc.alloc_semaphore("s_dg")
        s_db = nc.alloc_semaphore("s_db")
        s_tr = nc.alloc_semaphore("s_tr")
        s_cpy = nc.alloc_semaphore("s_cpy")
        s_mm = nc.alloc_semaphore("s_mm")
        s_oc = nc.alloc_semaphore("s_oc")
        s_st = [nc.alloc_semaphore(f"s_st{i}") for i in range(2)]
        blk_cm = nc.Block()
        block = blk_cm.__enter__()

        @block.sync
        def _(sync):
            sync.dma_start(idx_sb[:], idx_d[:]).then_inc(s_ix, 16)
            sync.dma_start(wt_sb[:], wt.rearrange("t c e -> c t e")).then_inc(s_iow, 16)
            for src, dst in [(wtl_d, wtl), (wtr_d, wtr), (wbl_d, wbl),
                             (wbr_d, wbr), (ident, id_sb)]:
                sync.dma_start(dst[:], src[:]).then_inc(s_io, 16)
            for g in range(N_GP):
                for ob in range(2):
                    j = 2 * g + ob
                    sync.wait_ge(s_oc, j + 1)
                    sync.dma_start(out[ob * 128:(ob + 1) * 128,
                                       g * GEMM_NP:(g + 1) * GEMM_NP],
                                   out_sb[ob][:]).then_inc(s_st[ob], 16)
            sync.wait_ge(s_st[0], 16 * N_GP)
            sync.wait_ge(s_st[1], 16 * N_GP)

        @block.vector
        def _(vector):
            vector.wait_ge(s_io, IO_ALL)
            for c in range(N_PB):
                if c >= NDB:
                    vector.wait_ge(s_tr, 5 * (c - NDB + 1))
                if "combine" in skip:
                    vector.drain().then_inc(s_dg, 1)
                    continue
                vector.wait_ge(s_g[c % NBUF], 32 * (c // NBUF + 1))
                d = dbuf[c % NDB]
                cbf = comb[c % NDB]
                gg = g_buf[c % NBUF]
                s0 = KK * c
                # sweep A: first corner of DVE taps + all diag builds
                for ki, k in enumerate(DVE_TAPS):
                    vector.tensor_scalar(cbf[:, ki, :], gg[:, k, 0:256],
                                         wtl[:, s0 + k:s0 + k + 1], None, AL.mult)
                for k in range(KK):
                    if k in DVE_TAPS or k in ACT_TAPS:
                        continue
                    for j2, w_ in ((0, wtl), (1, wtr), (2, wbl), (3, wbr)):
                        vector.tensor_scalar(d[:, j2, k, :], id_sb[:],
                                             w_[:, s0 + k:s0 + k + 1], None, AL.mult)
                for off, w_ in ((512, wtr), (256, wbl), (768, wbr)):
                    for ki, k in enumerate(DVE_TAPS):
                        vector.scalar_tensor_tensor(
                            cbf[:, ki, :], gg[:, k, off:off + 256],
                            w_[:, s0 + k:s0 + k + 1], cbf[:, ki, :],
                            AL.mult, AL.add)
                vector.drain().then_inc(s_dg, 1)

        @block.gpsimd
        def _(gpsimd):
            gpsimd.load_library(library_config.mlp)
            gpsimd.wait_ge(s_ix, 16)
            zt_view = zt[:].copy()
            zt_view.ap = bass_rust.VecI64Pair([[2 * C, NZT], [1, 4 * C]])
            for c in range(N_PB):
                if c >= NBUF:
                    gpsimd.wait_ge(s_tr, 5 * (c - NBUF + 1))
                i0 = c * 72
                # one dma_gather tops out at 1024 idx -> slot-aligned split
                # 1152 = 512 + 640; queue assignment rotates per chunk.
                for qi, (lo, hi, sl0, sl1) in enumerate(((0, 32, 0, 4),
                                                        (32, 72, 4, 9))):
                    nidx = (hi - lo) * 16
                    if "gather" in skip:
                        gpsimd.sem_inc(s_g[c % NBUF], 16)
                        continue
                    gpsimd.dma_gather(g_buf[c % NBUF][:, sl0:sl1, :], zt_view,
                                      idx_sb[:, i0 + lo:i0 + hi],
                                      nidx, nidx, 4 * C, elem_step=2 * C,
                                      queue_num=(2 * qi + c) % 4).then_inc(
                                          s_g[c % NBUF], 16)

        @block.tensor
        def _(tensor):
            tensor.wait_ge(s_io, IO_ALL)
            tensor.wait_ge(s_iow, 16)
            for c in range(N_PB):
                tensor.wait_ge(s_dg, c + 1)
                tensor.wait_ge(s_db, c + 1)
                tensor.wait_ge(s_g[c % NBUF], 32 * (c // NBUF + 1))
                gg = g_buf[c % NBUF]
                d = dbuf[c % NDB]
                cbf = comb[c % NDB]
                for k in range(KK):
                    for cb in range(2):
                        t = 2 * k + cb
                        g4, q = t // 4, t % 4
                        jj = c * 5 + g4          # global 4-group index
                        if "pe" in skip:
                            if q == 0:
                                tensor.sem_inc(s_tr, 1)
                            continue
                        if q == 0 and jj >= NPST:
                            tensor.wait_ge(s_cpy, jj - NPST + 1)
                        if k in DVE_TAPS:
                            ki = DVE_TAPS.index(k)
                            mm = tensor.matmul(ps_t[jj % NPST][:, q, :],
                                               cbf[:, ki, cb * 128:(cb + 1) * 128],
                                               id_sb[:], start=True, stop=True)
                        else:
                            for ci in range(4):
                                src = gg[:, k, W_OFFS[ci] + cb * 128:
                                         W_OFFS[ci] + (cb + 1) * 128]
                                mm = tensor.matmul(ps_t[jj % NPST][:, q, :], src,
                                                   d[:, ci, k, :],
                                                   start=(ci == 0), stop=(ci == 3))
                        if q == 3 or t == N_CKT - 1:
                            mm.then_inc(s_tr, 1)
                if c % 4 == 3:
                    g = c // 4
                    tensor.wait_ge(s_cpy, 5 * (c + 1))
                    for ob in range(2):
                        if "pe" in skip:
                            tensor.sem_inc(s_mm, 1)
                            continue
                        if g >= 1:
                            tensor.wait_ge(s_oc, 2 * (g - 1) + ob + 1)
                        for t in range(N_CKT):
                            mm = tensor.matmul(
                                ps_g[ob][:],
                                wt_sb[:, t, ob * 128:(ob + 1) * 128],
                                cols[:, t, g * GEMM_NP:(g + 1) * GEMM_NP],
                                start=(t == 0), stop=(t == N_CKT - 1))
                        mm.then_inc(s_mm, 1)

        @block.scalar
        def _(scalar):
            scalar.wait_ge(s_io, IO_ALL)
            for c in range(N_PB):
                if "act" in skip:
                    scalar.sem_inc(s_db, 1)
                else:
                    d = dbuf[c % NDB]
                    s0 = KK * c
                    for ti_, k in enumerate(ACT_TAPS):
                        for j2, w_ in ((0, wtl), (1, wtr), (2, wbl), (3, wbr)):
                            a = scalar.activation(d[:, j2, k, :], id_sb[:],
                                                  AF.Copy,
                                                  scale=w_[:, s0 + k:s0 + k + 1])
                    a.then_inc(s_db, 1)
                for g4 in range(5):
                    nt = 4 if g4 < 4 else 2      # tiles in this group (18 = 4*4+2)
                    jj = c * 5 + g4
                    if "act" in skip:
                        scalar.sem_inc(s_cpy, 1)
                        continue
                    scalar.wait_ge(s_tr, jj + 1)
                    t0_ = g4 * 4
                    scalar.activation(cols[:, t0_:t0_ + nt, c * 128:(c + 1) * 128],
                                      ps_t[jj % NPST][:, 0:nt, :],
                                      AF.Copy).then_inc(s_cpy, 1)
                if c % 4 == 3:
                    g = c // 4
                    for ob in range(2):
                        jj = 2 * g + ob
                        if "act" in skip:
                            scalar.sem_inc(s_oc, 1)
                            continue
                        scalar.wait_ge(s_mm, jj + 1)
                        if g >= 1:
                            scalar.wait_ge(s_st[ob], 16 * g)
                        scalar.activation(out_sb[ob][:], ps_g[ob][:],
                                          AF.Copy).then_inc(s_oc, 1)

        blk_cm.__exit__(None, None, None)

    snap = nc._state.snapshot_sems()
    for rep in range(reps):
        emit_body()
        if rep < reps - 1:
            nc.clear_and_free_semaphores(nc._state.allocated_since(snap))
            nc.all_engine_barrier()
            nc._state.restore_sems(snap)

    st.close()
    nc.compile()
    return nc


def _to_w(a, dtype):
    # w-layout [128, N_SLOT]: (part, KK*pb + k) = val[k, pb*128 + part]
    return np.ascontiguousarray(
        a.reshape(KK, N_PB, 128).transpose(2, 1, 0)).reshape(128, N_SLOT).astype(dtype)


def _to_g(a, dtype):
    # g-layout [128, NG]: (16g + l, 72*pb + 8*k + w) = val[k, pb*128 + w*16 + l]
    a4 = a.reshape(KK, N_PB, 8, 16)              # [k, pb, w, l]
    g1 = np.ascontiguousarray(a4.transpose(3, 1, 0, 2)).reshape(16, NG)
    return np.tile(g1, (8, 1)).astype(dtype)     # replicate to 128 partitions


def _host_prep(x, offset, weight):
    """Build the 8 per-core input maps."""
    f16 = np.float16
    # row-pair duplicated image zt[y, x] = (x[:, y, x], x[:, y+1, x]), fp16,
    # with a zero column at x=W and zero second half at y=H-1.
    zts = []
    for b in range(B):
        t = np.ascontiguousarray(x[b].transpose(1, 2, 0)).astype(f16)  # [H, W, C]
        zrow = np.zeros((H, W + 1, 2 * C), dtype=f16)
        zrow[:, :W, :C] = t
        zrow[:H - 1, :W, C:] = t[1:]
        ztf = np.zeros(((NZT + 1) * 2 * C,), dtype=f16)
        ztf[:NZT * 2 * C] = zrow.reshape(-1)
        zts.append(ztf)
    # weights: wt[t, c, o] = weight[o, cb*128+c, ky, kx],  t = 2*(3*ky+kx) + cb
    wr = weight.reshape(O, C, KK).transpose(2, 1, 0)   # [KK, C, O]
    wt = np.empty((N_CKT, 128, O), dtype=f16)
    for k in range(KK):
        for cb in range(2):
            wt[2 * k + cb] = wr[k, cb * 128:(cb + 1) * 128, :].astype(f16)

    ident = np.eye(128, dtype=f16)

    ky, kx = np.meshgrid(np.arange(3), np.arange(3), indexing="ij")
    ky = ky.reshape(-1).astype(np.float64)   # [KK]
    kx = kx.reshape(-1).astype(np.float64)

    in_maps = []
    p = np.arange(P_CORE)
    for core in range(N_CORES):
        b, hhalf = core // 2, core % 2
        i0 = hhalf * ROWS_PER_CORE
        off = offset[b].reshape(KK, 2, H, W)[:, :, i0:i0 + ROWS_PER_CORE, :]
        offy = off[:, 0].reshape(KK, P_CORE).astype(np.float64)
        offx = off[:, 1].reshape(KK, P_CORE).astype(np.float64)
        py = (i0 + p // W - 1)[None, :] + ky[:, None] + offy   # [KK, P]
        px = (p % W - 1)[None, :] + kx[:, None] + offx
        y0 = np.floor(py)
        x0 = np.floor(px)
        dy = py - y0
        dx = px - x0
        # slot weights: boundary-shifted so the 2x2 window at the clipped
        # entry (yc, xc) carries exactly the valid reference corner weights
        vy0 = (y0 >= 0) & (y0 <= H - 1)
        vy1 = (y0 + 1 >= 0) & (y0 + 1 <= H - 1)
        wy0 = (1.0 - dy) * vy0 + dy * (vy1 & (y0 == -1))
        wy1 = dy * (vy1 & (y0 != -1))
        vx0 = (x0 >= 0) & (x0 <= W - 1)
        vx1 = (x0 + 1 >= 0) & (x0 + 1 <= W - 1)
        wx0 = (1.0 - dx) * vx0 + dx * (vx1 & (x0 == -1))
        wx1 = dx * (vx1 & (x0 != -1))
        yc = np.clip(y0, 0, H - 1).astype(np.int64)
        xc = np.clip(x0, 0, W - 1).astype(np.int64)
        idx = (yc * (W + 1) + xc).astype(np.int16)

        in_maps.append({
            "zt": zts[b], "wt": wt, "ident": ident,
            "idx": _to_g(idx, np.int16),
            "wtl": _to_w(wy0 * wx0, np.float32), "wtr": _to_w(wy0 * wx1, np.float32),
            "wbl": _to_w(wy1 * wx0, np.float32), "wbr": _to_w(wy1 * wx1, np.float32),
        })
    return in_maps


def _assemble(results):
    out = np.empty((B, O, H, W), dtype=np.float32)
    for core in range(N_CORES):
        b, hhalf = core // 2, core % 2
        i0 = hhalf * ROWS_PER_CORE
        out[b, :, i0:i0 + ROWS_PER_CORE, :] = \
            np.asarray(results[core]["out"]).astype(np.float32).reshape(
                O, ROWS_PER_CORE, W)
    return out


def _make_exec(nc, donate=False):
    """Build a cached jitted SPMD executor for a compiled Bass module.

    Replicates concourse.bass2jax.run_bass_via_pjrt's lowering (same
    _bass_exec_p bind / shard_map layout) but returns a reusable jitted
    callable, so repeated invocations skip re-trace/re-lower/re-compile.
    """
    import jax
    import numpy as _np
    from jax.sharding import Mesh, PartitionSpec
    from jax.experimental.shard_map import shard_map
    from concourse import bass2jax
    import concourse.mybir as mybir

    bass2jax.install_neuronx_cc_hook()
    assert nc.dbg_addr is None
    partition_name = (nc.partition_id_tensor.name
                      if nc.partition_id_tensor else None)

    in_names, out_names, out_avals, zero_outs = [], [], [], []
    for alloc in nc.m.functions[0].allocations:
        if not isinstance(alloc, mybir.MemoryLocationSet):
            continue
        name = alloc.memorylocations[0].name
        if alloc.kind == "ExternalInput":
            if name != partition_name:
                in_names.append(name)
        elif alloc.kind == "ExternalOutput":
            out_names.append(name)
            shape = tuple(alloc.tensor_shape)
            dtype = mybir.dt.np(alloc.dtype)
            out_avals.append(jax.core.ShapedArray(shape, dtype))
            zero_outs.append(_np.zeros(shape, dtype))
    n_params = len(in_names)
    all_names = list(in_names) + list(out_names)
    if partition_name is not None:
        all_names.append(partition_name)
    all_names = tuple(all_names)

    def _body(*args):
        operands = list(args)
        if partition_name is not None:
            operands.append(bass2jax.partition_id_tensor())
        outs = bass2jax._bass_exec_p.bind(
            *operands,
            out_avals=tuple(out_avals),
            in_names=all_names,
            out_names=tuple(out_names),
            lowering_input_output_aliases=(),
            sim_require_finite=True,
            sim_require_nnan=True,
            nc=nc,
        )
        return tuple(outs)

    devices = jax.devices()[:N_CORES]
    mesh = Mesh(np.asarray(devices), ("core",))
    n_out = len(out_names)
    fn = jax.jit(
        shard_map(_body, mesh=mesh,
                  in_specs=(PartitionSpec("core"),) * (n_params + n_out),
                  out_specs=(PartitionSpec("core"),) * n_out,
                  check_rep=False),
        donate_argnums=tuple(range(n_params, n_params + n_out)) if donate else (),
        keep_unused=True,
    )
    return {"fn": fn, "in_names": in_names, "out_names": out_names,
            "zero_outs": zero_outs, "mesh": mesh, "n_params": n_params}


def _concat_inputs(ex, in_maps):
    return [np.concatenate([in_maps[c][n] for c in range(N_CORES)], axis=0)
            for n in ex["in_names"]]


def _concat_zeros(ex):
    return [np.zeros((N_CORES * z.shape[0], *z.shape[1:]), z.dtype)
            for z in ex["zero_outs"]]


def kernel(x, offset, weight):
    x = np.asarray(x, dtype=np.float32)
    offset = np.asarray(offset, dtype=np.float32)
    weight = np.asarray(weight, dtype=np.float32)
    if "nc" not in _CACHE:
        _CACHE["nc"] = _build_nc()
    if "exec" not in _CACHE:
        _CACHE["exec"] = _make_exec(_CACHE["nc"])
    ex = _CACHE["exec"]
    in_maps = _host_prep(x, offset, weight)
    outs = ex["fn"](*_concat_inputs(ex, in_maps), *_concat_zeros(ex))
    full = np.asarray(outs[0]).reshape(N_CORES, O, P_CORE)
    results = [{"out": full[c]} for c in range(N_CORES)]
    return _assemble(results)
